# revision 60
# baseline (speedup 1.0000x reference)
"""NonLocalBlock (self-attention over 64x64 image, C=256, D=32) on 8 trn2 cores.

Sharding: data-parallel over B=4 batches x 2-way split of the attention
rows (the `n` axis of beta[n, m]) => 8 cores, each computing a [2048, 256]
slice of the output. Each core receives its batch image pre-transposed
(and fp16-cast) by the host, rolled so its own 2048 rows come first,
plus its own half in natural layout for the residual. The host also
pre-casts the 1x1-conv weights and folds gamma into Wv.

Device math (per core, n = its 2048 key rows, m = all 4096 queries):
  logits[m, n] = q_m . k_n               PE, fp16, [128m x 512n] matmuls
  E[m, n] = exp(logits)                  ACT exact exp + DVE exp2 bit-trick
                                         (int16(l*128*log2e + 16250.875)
                                          bitcast to bf16, trunc-fitted)
  o[n, :] = sum_m E[m, n] v_aug[m, :]    PE, E stationary (33 cols/matmul),
                                         terms issued a few steps behind
                                         each exp tile; 8 chains share one
                                         PSUM zero-region (single start/stop)
  obar = o[:, 0:32] / o[:, 32]           DVE reciprocal + ACT/DVE scale, bf16
  oT = transpose(obar)                   PE (identity-moving), 32-row bands
  out[n, :] = oT.T @ (gamma Wv) + x      PE matmul + {ACT copy + Pool add |
                                         DVE fused add}, DMA out

Engine layout: ACT exp tiles are [128,1024] double-buffered in PSUM
(2x2 banks), DVE trick tiles are [128,512] double-buffered (2x1 banks),
o-chain accumulator 1 bank, final tiles 1 bank.
"""

from contextlib import ExitStack

import ml_dtypes
import numpy as np

import concourse.bass as bass
import concourse.tile as tile
from concourse import bacc, mybir
from concourse.bass_utils import run_bass_kernel_spmd

B, H, W, C = 4, 64, 64, 256
N = H * W            # 4096 pixels per image
D = 32               # reduced channel dim
NH = N // 2          # key rows owned by each core
P = 128
MT = N // P          # 32 query (m) tiles
NT = NH // P         # 16 n-tiles of 128 per core
SG = 2               # supergroups of 1024 n-columns
SGW = NH // SG       # 1024
FP32 = mybir.dt.float32
BF16 = mybir.dt.bfloat16
FP16 = mybir.dt.float16
I16 = mybir.dt.int16
NCORES = 8

# exp(l) ~= bf16-bitcast(int16(l * 128*log2(e) + 16250.875)); the int16
# convert truncates, constant fitted for that (max rel err 3.3%)
EXP_S1 = float(np.float32(128 * 1.4426950408889634))
EXP_S2 = 16250.875
Aop = mybir.AluOpType

LAST_RESULTS = None  # BassKernelResults of the most recent run (for test.py)

LAG = 6  # steps between an exp tile and its o-chain consumption


def _exp_pattern(n_act=19, n_dve=13):
    """Weighted round-robin ACT/DVE assignment for exp tiles (per 32)."""
    counts = {"A": float(n_act), "D": float(n_dve)}
    total = sum(counts.values())
    acc = dict.fromkeys(counts, 0.0)
    seq = []
    for _ in range(int(total)):
        for k in counts:
            acc[k] += counts[k] / total
        pick = max(acc, key=lambda k: acc[k])
        acc[pick] -= 1.0
        seq.append(pick)
    return seq


def _body(ctx, tc, out_d, xh_d, xt_d, wf_d, wg_d, wh_d, wv_d, id_d):
    nc = tc.nc
    const = ctx.enter_context(tc.tile_pool(name="const", bufs=1))
    big = ctx.enter_context(tc.tile_pool(name="big", bufs=1))
    ep = ctx.enter_context(tc.tile_pool(name="ep", bufs=32))
    obp = ctx.enter_context(tc.tile_pool(name="obp", bufs=6))
    otp = ctx.enter_context(tc.tile_pool(name="otp", bufs=6))
    spp = ctx.enter_context(tc.tile_pool(name="spp", bufs=2))
    ocp = ctx.enter_context(tc.tile_pool(name="ocp", bufs=2))
    rcp = ctx.enter_context(tc.tile_pool(name="rcp", bufs=6))
    fin = ctx.enter_context(tc.tile_pool(name="fin", bufs=16))
    psA = ctx.enter_context(tc.tile_pool(name="psA", bufs=2, space="PSUM"))
    psD = ctx.enter_context(tc.tile_pool(name="psD", bufs=3, space="PSUM"))
    psO = ctx.enter_context(tc.tile_pool(name="psO", bufs=1, space="PSUM"))

    # ---- tiny weights first (instant transfers) on the ACT HWDGE queue ----
    w_sb = {}
    for name, wd in (("f", wf_d), ("g", wg_d), ("h", wh_d)):
        wb = const.tile([P, 2, D], FP16, tag=f"w{name}")
        nc.scalar.dma_start(wb[:], wd.rearrange("c p d -> p c d"))
        w_sb[name] = wb
    wvr = const.tile([P, C], BF16)
    nc.scalar.dma_start(wvr[:], wv_d)
    ident = const.tile([P, P], BF16)
    nc.scalar.dma_start(ident[:], id_d)

    xt = big.tile([P, 2, N], FP16)  # xT: [c (2 chunks of 128), m]
    pieces = [(0, 512), (512, 1024), (1024, 2048), (2048, 3072), (3072, 4096)]
    for a, b in pieces:
        for ch in range(2):
            nc.sync.dma_start(xt[:, ch, a:b], xt_d[ch, :, a:b])
    x_half = big.tile([P, NT, C], FP32)
    qt = big.tile([D, N], FP16)            # q: [d, m]
    kt = big.tile([D, NH], FP16)           # k: [d, n] (own half only)
    v_sb = big.tile([P, MT, D + 1], BF16)  # v: [m, d | 1]
    nc.vector.memset(v_sb[:, :, D:D + 1], 1.0)

    def proj_mm(w, mg, nm):
        pp = psD.tile([D, 512], FP32, tag="pd", name=f"p{nm}{mg}")
        for ch in range(2):
            nc.tensor.matmul(
                pp[:], w[:, ch, :], xt[:, ch, mg * 512:(mg + 1) * 512],
                start=(ch == 0), stop=(ch == 1),
            )
        return pp

    def proj_copy(pp, dst, mg, on_act=False):
        if on_act:
            nc.scalar.copy(dst[:, mg * 512:(mg + 1) * 512], pp[:])
        else:
            nc.vector.tensor_copy(dst[:, mg * 512:(mg + 1) * 512], pp[:])

    def proj(w, dst, mg, nm, on_act=False):
        proj_copy(proj_mm(w, mg, nm), dst, mg, on_act)

    def v_mm(mtg):
        pv = psA.tile([P, 4, D], FP32, tag="pa", name=f"pv{mtg}")
        for j in range(4):
            mt = mtg * 4 + j
            for ch in range(2):
                nc.tensor.matmul(
                    pv[:, j, :], xt[:, ch, mt * P:(mt + 1) * P],
                    w_sb["h"][:, ch, :],
                    start=(ch == 0), stop=(ch == 1),
                )
        return pv

    def v_copy(pv, mtg, on_act=False):
        if on_act:
            nc.scalar.copy(v_sb[:, mtg * 4:(mtg + 1) * 4, 0:D], pv[:])
        else:
            nc.vector.tensor_copy(v_sb[:, mtg * 4:(mtg + 1) * 4, 0:D], pv[:])

    def v_batch(mtg, on_act=False):
        v_copy(v_mm(mtg), mtg, on_act)

    # PE p-state warmup: tiny matmuls on a memset tile (no DMA dependency)
    wsrc = big.tile([P, D], BF16, tag="wsrc")
    nc.vector.memset(wsrc[:], 0.25)
    warm = psA.tile([P, 64], FP32, tag="pa", name="warm")
    for _ in range(32):
        nc.tensor.matmul(
            warm[0:D, 0:D], wsrc[:], wsrc[:, 0:D],
            start=True, stop=True, skip_group_check=True,
        )
    nc.vector.tensor_copy(v_sb[0:D, 0, 0:D], warm[0:D, 0:D])  # keep it live

    # prologue projections (ACT is otherwise idle this early)
    pq0 = proj_mm(w_sb["f"], 0, "q")
    pk0 = proj_mm(w_sb["g"], 0, "k")
    proj_copy(pq0, qt, 0, on_act=True)
    proj_copy(pk0, kt, 0, on_act=False)
    pq1 = proj_mm(w_sb["f"], 1, "q")
    pk1 = proj_mm(w_sb["g"], 1, "k")
    proj_copy(pq1, qt, 1, on_act=True)
    proj_copy(pk1, kt, 1, on_act=False)
    v_batch(0, on_act=True)
    xh_src = xh_d.rearrange("(s p) c -> p s c", p=P)

    pat0 = _exp_pattern(19, 13)   # sg0: DVE busy with staged proj copies
    pat1 = _exp_pattern(19, 13)   # sg1: DVE freer

    def o_mms(sg, j, oB, ets):
        # one accumulation term (query tile j) for all 8 chains of the
        # supergroup; the chains share one PSUM zero-region, so only the
        # very first matmul starts it and the very last stops it (bytes
        # zero lazily on first touch)
        for t in range(8):
            if len(ets) == 1:
                esrc = ets[0][:, t * P:(t + 1) * P]
            else:
                esrc = ets[t // 4][:, (t % 4) * P:(t % 4 + 1) * P]
            nc.tensor.matmul(
                oB[:, t, :], esrc, v_sb[:, j, :],
                start=(j == 0 and t == 0), stop=(j == MT - 1 and t == 7),
            )

    # ---- software-pipelined epilogue stages (issued >=1 step after deps) ----
    import collections
    sched = collections.defaultdict(list)

    def defer(step, fn):
        sched[step].append(fn)

    def emit_finals(sg, first_step, spread):
        """Stage the obars/transpose/final pipeline for supergroup sg.
        Each stage is issued `spread` steps after its producer so every
        instruction's deps are satisfied at issue time (no head-of-line
        blocking in the in-order engine queues)."""
        st = first_step
        ctx2 = {}

        def recs(oB):
            def f():
                rec = rcp.tile([P, 8], FP32, tag="rec", name=f"rec{sg}")
                nc.vector.reciprocal(rec[:], oB[:, :, D])
                ctx2["rec"] = rec
            return f

        def oc_copy(oB, h2):
            # raw (unnormalized) chain outputs -> SBUF bf16, one op per half
            def f():
                oc = obp.tile([P, 4, D], BF16, tag="ob", name=f"oc{sg}_{h2}")
                if h2 == 0:
                    nc.scalar.copy(oc[:], oB[:, h2 * 4:(h2 + 1) * 4, 0:D])
                else:
                    nc.vector.tensor_copy(oc[:], oB[:, h2 * 4:(h2 + 1) * 4, 0:D])
                ctx2[("oc", h2)] = oc
            return f

        def ot_mms(h2):
            # single matmul transposes all 4 bands: lhsT free dims (4, 32)
            # stack onto the 128 output partitions
            def f():
                oTps = psD.tile([P, P], FP32, tag="pd", name=f"otp{sg}_{h2}")
                nc.tensor.matmul(oTps[:], ctx2[("oc", h2)][:], ident[:],
                                 start=True, stop=True)
                ctx2[("otp", h2)] = oTps
            return f

        def ot_copy(h2):
            def f():
                oT = otp.tile([P, P], BF16, tag="ot", name=f"ot{sg}_{h2}")
                if h2 == 0:
                    nc.scalar.copy(oT[:], ctx2[("otp", h2)][:])
                else:
                    nc.vector.tensor_copy(oT[:], ctx2[("otp", h2)][:])
                ctx2[("ot", h2)] = oT
            return f

        def sp_move(h2):
            # band 3 lands at partition 96 which matmul lhsT cannot address;
            # transpose it again separately to a base-0 tile via PE
            def f():
                sp_ps = psD.tile([D, P], FP32, tag="pd", name=f"spp{sg}_{h2}")
                nc.tensor.matmul(sp_ps[:], ctx2[("oc", h2)][:, 3, :], ident[:],
                                 start=True, stop=True)
                sp = spp.tile([D, P], BF16, tag="sp", name=f"sp{sg}_{h2}")
                if h2 == 0:
                    nc.scalar.copy(sp[:], sp_ps[:])
                else:
                    nc.vector.tensor_copy(sp[:], sp_ps[:])
                ctx2[("sp", h2)] = sp
            return f

        def f_mm(h2, bd):
            def f():
                nt = sg * 8 + h2 * 4 + bd
                # tail finals can also use the (then idle) ACT-lane slots
                fpool, ftag = (psA, "pa") if (sg == 1 and bd % 2 == 1) else (psD, "pd")
                fps = fpool.tile([P, C], FP32, tag=ftag, name=f"F{nt}")
                if bd < 3:
                    nc.tensor.matmul(fps[:],
                                     ctx2[("ot", h2)][bd * D:(bd + 1) * D, :],
                                     wvr[bd * D:(bd + 1) * D, :],
                                     start=True, stop=True)
                else:
                    nc.tensor.matmul(fps[:], ctx2[("sp", h2)][:], wvr[0:D, :],
                                     start=True, stop=True)
                ctx2[("f", h2, bd)] = fps
            return f

        def f_scale_add(h2, bd):
            def f():
                nt = sg * 8 + h2 * 4 + bd
                t = h2 * 4 + bd
                fps = ctx2[("f", h2, bd)]
                rec = ctx2["rec"]
                osb = fin.tile([P, C], FP32, tag="osb", name=f"osb{nt}")
                if bd % 2 == 0:
                    nc.scalar.activation(osb[:], fps[:],
                                         mybir.ActivationFunctionType.Copy,
                                         scale=rec[:, t:t + 1])
                    nc.gpsimd.tensor_add(osb[:], osb[:], x_half[:, nt, :])
                else:
                    nc.vector.tensor_scalar(osb[:], fps[:], rec[:, t:t + 1],
                                            None, Aop.mult)
                    nc.vector.tensor_add(osb[:], osb[:], x_half[:, nt, :])
                ctx2[("osb", h2, bd)] = osb
            return f

        def f_dma(h2, bd):
            def f():
                nt = sg * 8 + h2 * 4 + bd
                osb = ctx2[("osb", h2, bd)]
                if sg == 1 and bd == 3:
                    dq = nc.gpsimd   # pool-queue: idle engine, parallel path
                elif sg == 1 and h2 == 1:
                    dq = nc.scalar
                else:
                    dq = nc.sync
                dq.dma_start(out_d[nt * P:(nt + 1) * P, :], osb[:])
            return f

        def f_out(h2, bd):
            def f():
                f_scale_add(h2, bd)()
                f_dma(h2, bd)()
            return f

        oB = oBs[sg]
        if spread == 0:
            # dense tail: interleave both halves level-by-level so their
            # stage chains run concurrently on different engines
            defer(st, recs(oB))
            for h2 in range(2):
                defer(st, oc_copy(oB, h2))
            for h2 in range(2):
                defer(st, ot_mms(h2))
            for h2 in range(2):
                defer(st, ot_copy(h2))
            for h2 in range(2):
                defer(st, sp_move(h2))
            for bd in range(4):
                for h2 in range(2):
                    defer(st, f_mm(h2, bd))
            for bd in range(4):
                for h2 in range(2):
                    defer(st, f_scale_add(h2, bd))
            for bd in range(4):
                for h2 in range(2):
                    defer(st, f_dma(h2, bd))
        else:
            defer(st, recs(oB))
            for h2 in range(2):
                b = st + (1 + h2 * 5) * spread
                defer(b, oc_copy(oB, h2))
                defer(b + spread, ot_mms(h2))
                defer(b + 2 * spread, ot_copy(h2))
                defer(b + 2 * spread, sp_move(h2))
                for bd in range(4):
                    defer(b + (3 + bd) * spread, f_mm(h2, bd))
                    defer(b + (4 + bd) * spread, f_out(h2, bd))

    # ---- main loop: flat over 64 beta/exp tiles, o-chains lag LAG steps ----
    oBs = [None, None]
    etiles = {}

    def o_step(s):
        sgp, j = divmod(s, MT)
        if j == 0:
            oBs[sgp] = psO.tile([P, 8, D + 1], FP32, tag="o", name=f"oB{sgp}")
        o_mms(sgp, j, oBs[sgp], etiles.pop(s))

    # x_half loads are only needed by the finals; put them on the sync queue
    # BEHIND the critical xt pieces (queues dispatch strictly in order)
    for piece in range(4):
        nc.sync.dma_start(
            x_half[:, piece * 4:(piece + 1) * 4, :],
            xh_src[:, piece * 4:(piece + 1) * 4, :],
        )

    # staged projection work: (kind, idx, mm_step); copy issues next step.
    # deadlines: q mg by step 4*mg-1; v mtg by step 4*mtg+LAG-1; k by 31
    stages = [
        ("v", 1, 4), ("q", 2, 5),
        ("v", 2, 7), ("q", 3, 9),
        ("k", 2, 10), ("v", 3, 11), ("q", 4, 13),
        ("k", 3, 14), ("v", 4, 15), ("q", 5, 17),
        ("v", 5, 19), ("q", 6, 21),
        ("v", 6, 23), ("q", 7, 25),
        ("v", 7, 27),
    ]
    for kind, idx, st in stages:
        def mk(kind, idx):
            def mm():
                if kind == "q":
                    etiles[("p", kind, idx)] = proj_mm(w_sb["f"], idx, "q")
                elif kind == "k":
                    etiles[("p", kind, idx)] = proj_mm(w_sb["g"], idx, "k")
                else:
                    etiles[("p", kind, idx)] = v_mm(idx)

            def cp():
                pp = etiles.pop(("p", kind, idx))
                if kind == "q":
                    proj_copy(pp, qt, idx)
                elif kind == "k":
                    proj_copy(pp, kt, idx)
                else:
                    v_copy(pp, idx, on_act=True)
            return mm, cp
        mm, cp = mk(kind, idx)
        defer(st, mm)
        defer(st + 1, cp)

    # finals for sg0 run spread through sg1; sg1's run densely at the end
    emit_finals_done = [False, False]

    NSTEPS = MT * SG
    for s in range(NSTEPS):
        sg, mt = divmod(s, MT)
        lane = (pat0 if sg == 0 else pat1)[s % 32]
        if lane == "A":
            et = ep.tile([P, SGW], BF16, tag="e", name=f"e{s}")
            etiles[s] = (et,)
            pb = psA.tile([P, SGW], FP32, tag="pa", name=f"pb{s}")
            for hf in range(2):
                nc.tensor.matmul(
                    pb[:, hf * 512:(hf + 1) * 512],
                    qt[:, mt * P:(mt + 1) * P],
                    kt[:, sg * SGW + hf * 512:sg * SGW + (hf + 1) * 512],
                    start=True, stop=True,
                )
            nc.scalar.activation(et[:], pb[:],
                                 mybir.ActivationFunctionType.Exp)
        else:
            ets = []
            for hf in range(2):
                eh = ep.tile([P, 512], BF16, tag="e", name=f"e{s}_{hf}")
                pb = psD.tile([P, 512], FP32, tag="pd", name=f"pb{s}_{hf}")
                nc.tensor.matmul(
                    pb[:],
                    qt[:, mt * P:(mt + 1) * P],
                    kt[:, sg * SGW + hf * 512:sg * SGW + (hf + 1) * 512],
                    start=True, stop=True,
                )
                nc.vector.tensor_scalar(
                    eh[:].bitcast(I16), pb[:],
                    EXP_S1, EXP_S2, Aop.mult, Aop.add)
                ets.append(eh)
            etiles[s] = tuple(ets)
        if s >= LAG:
            o_step(s - LAG)
            if s - LAG == MT - 1 and not emit_finals_done[0]:
                emit_finals(0, s + 1, 2)
                emit_finals_done[0] = True
        for fn in sched.pop(s, []):
            fn()
    # tail: trailing o-steps, then sg1 finals densely
    for s in range(NSTEPS - LAG, NSTEPS):
        o_step(s)
    emit_finals(1, NSTEPS, 0)
    for st in sorted(sched):
        for fn in sched.pop(st):
            fn()


def build_program():
    nc = bacc.Bacc(
        "TRN2",
        target_bir_lowering=False,
        debug=False,
        enable_asserts=False,
        num_devices=NCORES,
    )
    xh_d = nc.dram_tensor("xh", [NH, C], FP32, kind="ExternalInput").ap()
    xt_d = nc.dram_tensor("xt", [2, P, N], FP16, kind="ExternalInput").ap()
    wf_d = nc.dram_tensor("Wf16", [2, P, D], FP16, kind="ExternalInput").ap()
    wg_d = nc.dram_tensor("Wg16", [2, P, D], FP16, kind="ExternalInput").ap()
    wh_d = nc.dram_tensor("Wh16", [2, P, D], FP16, kind="ExternalInput").ap()
    wv_d = nc.dram_tensor("WvG", [P, C], BF16, kind="ExternalInput").ap()
    id_d = nc.dram_tensor("Ident", [P, P], BF16, kind="ExternalInput").ap()
    out_d = nc.dram_tensor("out", [NH, C], FP32, kind="ExternalOutput").ap()

    with tile.TileContext(nc) as tc:
        with ExitStack() as ctx:
            _body(ctx, tc, out_d, xh_d, xt_d, wf_d, wg_d, wh_d, wv_d, id_d)
    nc.compile()
    return nc


_CACHE = {}


def _get_program():
    if "nc" not in _CACHE:
        _CACHE["nc"] = build_program()
    return _CACHE["nc"]


def make_in_maps(inputs):
    x = np.ascontiguousarray(np.asarray(inputs["x"], np.float32)).reshape(B, N, C)
    gam = np.float32(np.asarray(inputs["gamma"], np.float32).reshape(()))
    w16 = {}
    for nm in ("Wf", "Wg", "Wh"):
        w = np.asarray(inputs[nm], np.float32).astype(np.float16)  # [256, 32]
        w16[nm] = np.ascontiguousarray(w.reshape(2, P, D))
    wv1 = (gam * np.asarray(inputs["Wv"], np.float32)).astype(ml_dtypes.bfloat16)
    wv = np.ascontiguousarray(np.tile(wv1, (4, 1)))
    ident = np.ascontiguousarray(np.eye(P, dtype=ml_dtypes.bfloat16))

    in_maps = []
    for c in range(NCORES):
        b, h = divmod(c, 2)
        if h == 0:
            xb = x[b]
        else:
            xb = np.concatenate([x[b, NH:], x[b, :NH]], axis=0)
        xt = np.ascontiguousarray(xb.T.astype(np.float16).reshape(2, P, N))
        in_maps.append(
            {
                "xh": np.ascontiguousarray(xb[:NH]),
                "xt": xt,
                "Wf16": w16["Wf"],
                "Wg16": w16["Wg"],
                "Wh16": w16["Wh"],
                "WvG": wv,
                "Ident": ident,
            }
        )
    return in_maps


def kernel(**inputs):
    global LAST_RESULTS
    nc = _get_program()
    in_maps = make_in_maps(inputs)
    res = run_bass_kernel_spmd(nc, in_maps, core_ids=list(range(NCORES)))
    LAST_RESULTS = res
    out = np.empty((B, N, C), np.float32)
    for c in range(NCORES):
        b, h = divmod(c, 2)
        out[b, h * NH:(h + 1) * NH] = res.results[c]["out"]
    return out.reshape(B, H, W, C)
